# revision 1
# baseline (speedup 1.0000x reference)
"""AttentionPooling Trainium2 kernel (8 NeuronCores, Bass/Tile).

Sharding: (batch, head-group) — core c handles batch b=c//2 and heads
4*(c%2)..4*(c%2)+3. Each core computes, for its 4 heads, Q^T/K^T (head-dim
major) and V (token major) projections, then a one-pass pooled attention:

  For each query stripe of 128 rows:  S = Q_stripe K^T / sqrt(d)  (PE, bf16)
  E = exp(S) (ScalarE), Z = rowsum(E) (VectorE), r = 1/Z (VectorE)
  w += r^T E (PE)   -- w[k] = sum_q E[q,k]/Z_q, PSUM-accumulated

  attended_mean * N = w @ V  (per head), then
  pooled_partial = concat_h(attended) @ (Wo_slice^T / N)

The mean-pool is folded through the output projection (linear), so the
(B,N,HID) attention output and the attn@V matmul are never materialized.
The V bias and output bias are folded on the host:
  pooled = pooled_partial(core even) + pooled_partial(core odd) + Wo@bv + bo

Pipelining: only head 0's Q/K projection runs as a prologue; the remaining
heads' Q/K and all V-projection matmuls are interleaved between attention
stripes so the TensorEngine fills the slack of the ScalarE-bound softmax.
PSUM budget (8 banks): S-stripe halves 2x[128,1024]f32 (4) + w accumulator
[128,1024]f32 (2, four (bank, partition-offset) sub-regions via matmul
column tile_position) + projection chunks 2x[128,512]f32 (2).

The host pre-transposes/casts the per-core operands (x[b]^T, W^T slices) so
the device does no transposes on the critical path; inputs are cast to bf16
(matmuls run at full PE rate; accumulation is fp32 in PSUM).
"""

import sys

import numpy as np

for _p in ("/opt/trn_rl_repo",):
    if _p not in sys.path:
        sys.path.append(_p)

import ml_dtypes

B, N, HID = 4, 2048, 1024
HEADS, HD = 8, 128
NH = 4          # heads per core
HGW = NH * HD   # head-group width (512)
NCORES = 8
P = 128
IT = HID // P   # 8 i-tiles
QT_TILES = N // P    # 16 query stripes
TOK_TILES = N // P   # 16 token tiles

BF16 = ml_dtypes.bfloat16

_cache = {}


def _build_nc():
    import concourse.bacc as bacc
    import concourse.tile as tile
    from concourse import mybir
    from concourse.bass import ds, ts
    from concourse.masks import make_identity
    from concourse.tile import add_dep_helper

    BF = mybir.dt.bfloat16
    F32 = mybir.dt.float32
    AF = mybir.ActivationFunctionType
    AX = mybir.AxisListType

    nc = bacc.Bacc(trn_type="TRN2")

    xT_d = nc.dram_tensor("xT", (HID, N), BF, kind="ExternalInput").ap()
    wqT_d = nc.dram_tensor("wqT", (NH, HID, HD), BF, kind="ExternalInput").ap()
    wkT_d = nc.dram_tensor("wkT", (NH, HID, HD), BF, kind="ExternalInput").ap()
    wvT_d = nc.dram_tensor("wvT", (HID, HGW), BF, kind="ExternalInput").ap()
    woT_d = nc.dram_tensor("woT", (HGW, HID), BF, kind="ExternalInput").ap()
    bq_d = nc.dram_tensor("bq_col", (P, NH), F32, kind="ExternalInput").ap()
    bk_d = nc.dram_tensor("bk_col", (P, NH), F32, kind="ExternalInput").ap()
    out_d = nc.dram_tensor("out_pooled", (1, HID), F32, kind="ExternalOutput").ap()

    inv_sqrt_d = float(1.0 / np.sqrt(HD))

    with tile.TileContext(nc) as tc:
        with (
            tc.tile_pool(name="persist", bufs=1) as persist,
            tc.tile_pool(name="sp", bufs=2, space="PSUM") as sp,
            tc.tile_pool(name="wp", bufs=1, space="PSUM") as wp,
            tc.tile_pool(name="pp", bufs=2, space="PSUM") as pp,
            tc.tile_pool(name="ep", bufs=3) as ep,
            tc.tile_pool(name="zp", bufs=4) as zp,
        ):
            # DMA order is the prologue critical path: head 0's Q/K weights
            # and the first token chunk of x^T land first so the first
            # projection matmuls start ~6us in; V/Wo/bias loads drain later
            # under the attention window.
            xT_sb = persist.tile([P, IT, N], BF)
            wq_sb = persist.tile([P, IT, NH, HD], BF)
            wk_sb = persist.tile([P, IT, NH, HD], BF)
            wv_sb = persist.tile([P, IT, HGW], BF)
            xT_r = xT_d.rearrange("(t p) n -> p t n", p=P)
            wqT_r = wqT_d.rearrange("h (t p) d -> h p t d", p=P)
            wkT_r = wkT_d.rearrange("h (t p) d -> h p t d", p=P)
            nc.sync.dma_start(out=wk_sb[:, :, 0, :], in_=wkT_r[0])
            nc.sync.dma_start(out=wq_sb[:, :, 0, :], in_=wqT_r[0])
            nc.sync.dma_start(out=wq_sb[:, :, 1, :], in_=wqT_r[1])
            nc.sync.dma_start(out=wk_sb[:, :, 1, :], in_=wkT_r[1])
            # x^T in two 2MiB halves: fewer per-DMA overheads, and the first
            # half's projection matmuls run while the second half transfers
            nc.sync.dma_start(out=xT_sb[:, : IT // 2, :], in_=xT_r[:, : IT // 2, :])
            nc.sync.dma_start(out=xT_sb[:, IT // 2 :, :], in_=xT_r[:, IT // 2 :, :])
            nc.sync.dma_start(
                out=wv_sb, in_=wvT_d.rearrange("(t p) d -> p t d", p=P)
            )
            for h in range(2, NH):
                nc.sync.dma_start(out=wq_sb[:, :, h, :], in_=wqT_r[h])
                nc.sync.dma_start(out=wk_sb[:, :, h, :], in_=wkT_r[h])
            wo_sb = persist.tile([P, NH, HID], BF)
            nc.sync.dma_start(out=wo_sb, in_=woT_d.rearrange("(t p) o -> p t o", p=P))
            bq_sb = persist.tile([P, NH], F32)
            bk_sb = persist.tile([P, NH], F32)
            nc.sync.dma_start(out=bq_sb, in_=bq_d)
            nc.sync.dma_start(out=bk_sb, in_=bk_d)
            ident = persist.tile([NH, NH], F32)
            make_identity(nc, ident)
            # one-hot columns: oneh_sb[p, h, h'] = 1.0 iff h == h'
            oneh_sb = persist.tile([P, NH, NH], BF)
            nc.vector.memset(oneh_sb, 0.0)
            for h in range(NH):
                nc.vector.memset(oneh_sb[:, h, h : h + 1], 1.0)
            zs4_sb = persist.tile([P, NH], BF)
            nc.vector.memset(zs4_sb, 0.0)

            QT_sb = persist.tile([P, NH, N], BF)
            KT_sb = persist.tile([P, NH, N], BF)
            V_sb = persist.tile([P, TOK_TILES, HGW], BF)
            w4_sb = persist.tile([NH, N], F32)
            # wTz[p, t, h, h'] = w_h[t*128+p] iff h' == h else 0 (block-diag
            # zero padding so per-head matmuls can emit 4-partition outputs)
            wTz_sb = persist.tile([P, TOK_TILES, NH, NH], BF)
            nc.vector.memset(wTz_sb, 0.0)
            att4_sb = persist.tile([NH, P], F32)
            attT_sb = persist.tile([P, NH], BF)
            pooled_sb = persist.tile([1, HID], F32)

            # last stripe-score matmul; background matmuls order behind it
            order_anchor = [None]

            def qk_chunk(proj_i, h, c, step=None, pool=None, tag="proj"):
                """One 512-token Q^T/K^T projection chunk for head h.
                As a generator (step=True) it yields after each 4-matmul
                half so background work interleaves in fine grains."""
                wsb, bsb, dst = (
                    (wq_sb, bq_sb, QT_sb),
                    (wk_sb, bk_sb, KT_sb),
                )[proj_i]
                ps = (pool or pp).tile([P, 512], F32, tag=tag, name="ps_qk")
                for i in range(IT):
                    mm = nc.tensor.matmul(
                        ps,
                        lhsT=wsb[:, i, h, :],
                        rhs=xT_sb[:, i, ts(c, 512)],
                        start=(i == 0),
                        stop=(i == IT - 1),
                    )
                    if False:
                        # keep background matmuls behind the latest stripe's
                        # score matmuls in the PE stream (scheduling-only dep;
                        # the greedy scheduler would otherwise front-load them
                        # and starve the ScalarE softmax pipeline)
                        add_dep_helper(
                            mm.ins, order_anchor[0].ins, sync=False, reason="bg-after-S"
                        )
                    if step and i == 3:
                        yield
                nc.vector.tensor_copy(dst[:, h, ts(c, 512)], ps)
                # per-partition bias (in-place, stride-0 free-dim broadcast)
                nc.vector.tensor_tensor(
                    dst[:, h, ts(c, 512)],
                    dst[:, h, ts(c, 512)],
                    bsb[:, h : h + 1].to_broadcast((P, 512)),
                    mybir.AluOpType.add,
                )
                if step:
                    yield

            def v_chunk(t, step=None):
                """One 128-token V projection tile (all 4 heads)."""
                ps = pp.tile([P, HGW], F32, tag="proj", name="ps_v")
                for i in range(IT):
                    mm = nc.tensor.matmul(
                        ps,
                        lhsT=xT_sb[:, i, ts(t, P)],
                        rhs=wv_sb[:, i, :],
                        start=(i == 0),
                        stop=(i == IT - 1),
                    )
                    if False:
                        add_dep_helper(
                            mm.ins, order_anchor[0].ins, sync=False, reason="bg-after-S"
                        )
                    if step and i == 3:
                        yield
                nc.vector.tensor_copy(V_sb[:, t, :], ps)
                if step:
                    yield

            # ---------------- prologue: head 0's K + first Q chunk --------
            # Stripe 0 needs all of K^T(h0) but only the first 128 queries of
            # Q^T(h0); the remaining Q chunks lead the background queue. The
            # five chunks borrow slots from all three PSUM pools so none of
            # them serializes on another's evacuation.
            for c, (pool_, tag_) in zip(
                range(4), ((pp, "proj"), (pp, "proj"), (sp, "s"), (sp, "s"))
            ):
                for _ in qk_chunk(1, 0, c, pool=pool_, tag=tag_):
                    pass
            for _ in qk_chunk(0, 0, 0, pool=wp, tag="w"):
                pass

            # Background projection work: remaining heads' Q/K and all V
            # tiles, emitted a few matmuls per stripe between the attention
            # matmul groups (the PE fills ScalarE-bound softmax slack).
            bg_tasks = []
            for c in range(1, 4):
                bg_tasks.append(qk_chunk(0, 0, c, step=True))
            for h2 in range(1, NH):
                for c in range(4):
                    bg_tasks.append(qk_chunk(0, h2, c, step=True))
                    bg_tasks.append(qk_chunk(1, h2, c, step=True))
                for t in range(NH * (h2 - 1), NH * h2):
                    bg_tasks.append(v_chunk(t, step=True))
            for t in range(NH * (NH - 1), NH * NH):
                bg_tasks.append(v_chunk(t, step=True))
            bg_tasks.reverse()  # consumed LIFO-from-front via pop() below
            BG_STEPS = 2 * len(bg_tasks)  # each generator yields twice
            BG_SPREAD = 48  # finish all background work by stripe 48 of 64

            def bg_advance(si):
                lo = si * BG_STEPS // BG_SPREAD
                hi = min((si + 1) * BG_STEPS // BG_SPREAD, BG_STEPS)
                for _ in range(max(0, hi - lo)):
                    while bg_tasks:
                        try:
                            next(bg_tasks[-1])
                            break
                        except StopIteration:
                            bg_tasks.pop()

            # ---------------- pooled attention ----------------
            # w accumulator: [128, 1024] fp32 = 2 PSUM banks. k-chunk j lives
            # at free range ts(j//2, 512), partitions [32*(j%2), +4) (heads on
            # partitions +0..3), via matmul column tile_position. Zero-matmuls
            # open each sub-region's accumulation group so later matmuls can
            # all use start=False regardless of has_written clear granularity.
            w4_ps = wp.tile([P, 1024], F32, tag="w", name="w4_ps")

            def w_region(j):
                poff = 32 * (j % 2)
                out = w4_ps[poff : poff + NH, ts(j // 2, 512)]
                tp = (0, poff) if poff else None
                return out, tp

            for j in range(4):
                out, tp = w_region(j)
                nc.tensor.matmul(
                    out,
                    lhsT=zs4_sb,
                    rhs=xT_sb[:, 0, ts(0, 512)],
                    start=True,
                    stop=False,
                    tile_position=tp,
                    skip_group_check=True,
                )

            def emit_S(h, qi):
                """Both k-half score matmul groups for one query stripe."""
                tiles = []
                for kk in range(2):
                    s_ps = sp.tile([P, 1024], F32, tag="s", name="s_ps")
                    for kc in range(2):
                        mm = nc.tensor.matmul(
                            s_ps[:, ts(kc, 512)],
                            lhsT=QT_sb[:, h, ts(qi, P)],
                            rhs=KT_sb[:, h, ds(kk * 1024 + kc * 512, 512)],
                            start=True,
                            stop=True,
                        )
                    tiles.append(s_ps)
                order_anchor[0] = mm
                return tiles

            def emit_w(pend, last):
                pe_, prb = pend
                for j in range(4):
                    out, tp = w_region(j)
                    nc.tensor.matmul(
                        out,
                        lhsT=prb,
                        rhs=pe_[:, ts(j, 512)],
                        start=False,
                        stop=last,
                        tile_position=tp,
                        skip_group_check=True,
                    )

            # Software-pipelined stripe loop: iteration (h, qi) consumes the
            # S tiles emitted in the previous iteration, emits the NEXT
            # stripe's S-matmuls first (so the exp chain never queues behind
            # other PE work), then the previous stripe's w-matmuls and a slice
            # of background projection work.
            pend_s = emit_S(0, 0)
            pend_w = None
            for h in range(NH):
                for qi in range(QT_TILES):
                    e_t = ep.tile([P, N], BF, tag="e", name="e_t")
                    zs = []
                    for kk, s_ps in enumerate(pend_s):
                        z_t = zp.tile([P, 1], F32, tag=f"z{kk}", name="z_t")
                        nc.scalar.activation(
                            out=e_t[:, ts(kk, 1024)],
                            in_=s_ps,
                            func=AF.Exp,
                            scale=inv_sqrt_d,
                            accum_out=z_t,
                        )
                        zs.append(z_t)
                    nqi = h * QT_TILES + qi + 1
                    if nqi < NH * QT_TILES:
                        pend_s = emit_S(nqi // QT_TILES, nqi % QT_TILES)
                    r_t = zp.tile([P, 1], F32, tag="r", name="r_t")
                    nc.vector.tensor_add(r_t, zs[0], zs[1])
                    nc.vector.reciprocal(r_t, r_t)
                    # rb4 column h = r (bf16), other columns zero
                    rb4_t = zp.tile([P, NH], BF, tag="rb", name="rb4_t")
                    nc.vector.tensor_tensor(
                        rb4_t,
                        oneh_sb[:, h, :],
                        r_t.to_broadcast((P, NH)),
                        mybir.AluOpType.mult,
                    )
                    if pend_w is not None:
                        emit_w(pend_w, False)
                    pend_w = (e_t, rb4_t)
                    # interleaved background projection work
                    bg_advance(h * QT_TILES + qi)
            emit_w(pend_w, True)

            # ---------------- tail: attended + output projection ----------
            for j in range(4):
                out, _ = w_region(j)
                nc.vector.tensor_copy(w4_sb[:, ts(j, 512)], out)
            for t in range(TOK_TILES):
                tp_ps = sp.tile([P, NH], F32, tag="s", name="tp_ps")
                nc.tensor.transpose(tp_ps, w4_sb[:, ts(t, P)], ident)
                # scatter into the block-diagonal (stride NH+1) positions
                nc.vector.tensor_copy(
                    wTz_sb[:, t].rearrange("p a b -> p (a b)")[:, :: NH + 1],
                    tp_ps,
                )
            att4_ps = sp.tile([NH, P], F32, tag="s", name="att4_ps")
            for t in range(TOK_TILES):
                for h in range(NH):
                    nc.tensor.matmul(
                        att4_ps,
                        lhsT=wTz_sb[:, t, h, :],
                        rhs=V_sb[:, t, ts(h, HD)],
                        start=(t == 0 and h == 0),
                        stop=(t == TOK_TILES - 1 and h == NH - 1),
                    )
            nc.vector.tensor_copy(att4_sb, att4_ps)
            attT_ps = sp.tile([P, NH], F32, tag="s", name="attT_ps")
            nc.tensor.transpose(attT_ps, att4_sb, ident)
            nc.vector.tensor_copy(attT_sb, attT_ps)
            p_ps = sp.tile([1, HID], F32, tag="s", name="p_ps")
            for oc in range(2):
                for h in range(NH):
                    nc.tensor.matmul(
                        p_ps[:, ts(oc, 512)],
                        lhsT=attT_sb[:, h : h + 1],
                        rhs=wo_sb[:, h, ts(oc, 512)],
                        start=(h == 0),
                        stop=(h == NH - 1),
                    )
            nc.vector.tensor_copy(pooled_sb, p_ps)
            nc.sync.dma_start(out=out_d, in_=pooled_sb)

    nc.finalize()  # Bacc: event-sem pass packs multi-waits into legal encodings
    return nc


def _get_nc():
    if "nc" not in _cache:
        _cache["nc"] = _build_nc()
    return _cache["nc"]


def _host_prep(inputs):
    """Build the 8 per-core input maps (host-side shard + transpose + cast)."""
    x = np.asarray(inputs["chunk_embeddings"], np.float32)
    in_maps = []
    for c in range(NCORES):
        b, hg = c // 2, c % 2
        sl = slice(hg * HGW, (hg + 1) * HGW)
        in_maps.append(
            {
                "xT": np.ascontiguousarray(x[b].T).astype(BF16),
                "wqT": np.ascontiguousarray(
                    np.asarray(inputs["Wq"], np.float32)[sl, :]
                    .T.reshape(HID, NH, HD)
                    .transpose(1, 0, 2)
                ).astype(BF16),
                "wkT": np.ascontiguousarray(
                    np.asarray(inputs["Wk"], np.float32)[sl, :]
                    .T.reshape(HID, NH, HD)
                    .transpose(1, 0, 2)
                ).astype(BF16),
                "wvT": np.ascontiguousarray(
                    np.asarray(inputs["Wv"], np.float32)[sl, :].T
                ).astype(BF16),
                "woT": np.ascontiguousarray(
                    np.asarray(inputs["Wo"], np.float32)[:, sl].T / np.float32(N)
                ).astype(BF16),
                "bq_col": np.ascontiguousarray(
                    np.asarray(inputs["bq"], np.float32)[sl].reshape(NH, P).T
                ),
                "bk_col": np.ascontiguousarray(
                    np.asarray(inputs["bk"], np.float32)[sl].reshape(NH, P).T
                ),
            }
        )
    return in_maps


def _unshard(results, inputs):
    bo = np.asarray(inputs["bo"], np.float32)
    bv = np.asarray(inputs["bv"], np.float32)
    Wo = np.asarray(inputs["Wo"], np.float32)
    bv_wo = Wo @ bv  # exact fold of the V bias through the output projection
    out = np.zeros((B, HID), np.float32)
    for b in range(B):
        out[b] = (
            results[2 * b]["out_pooled"][0]
            + results[2 * b + 1]["out_pooled"][0]
            + bv_wo
            + bo
        )
    return out


def _reference_numpy(inputs):
    """Fallback for non-trivial attention masks (never hit for the spec'd
    all-ones mask): straight numpy port of the reference."""
    x = np.asarray(inputs["chunk_embeddings"], np.float32)
    mask = np.asarray(inputs["attention_mask"])
    b, n, hid = x.shape

    def proj(W, bias):
        y = x @ np.asarray(W, np.float32).T + np.asarray(bias, np.float32)
        return y.reshape(b, n, HEADS, HD).transpose(0, 2, 1, 3)

    Q = proj(inputs["Wq"], inputs["bq"])
    K = proj(inputs["Wk"], inputs["bk"])
    V = proj(inputs["Wv"], inputs["bv"])
    s = np.einsum("bhqd,bhkd->bhqk", Q, K) / np.float32(np.sqrt(HD))
    s = np.where(mask[:, None, None, :] == 0, np.float32(-1e9), s)
    s = s - s.max(axis=-1, keepdims=True)
    e = np.exp(s)
    a = e / e.sum(axis=-1, keepdims=True)
    att = np.einsum("bhqk,bhkd->bhqd", a, V)
    att = att.transpose(0, 2, 1, 3).reshape(b, n, hid)
    out = att @ np.asarray(inputs["Wo"], np.float32).T + np.asarray(
        inputs["bo"], np.float32
    )
    m = mask[:, :, None].astype(np.float32)
    return (out * m).sum(axis=1) / m.sum(axis=1)


def _run(inputs, trace=False):
    from concourse.bass_utils import run_bass_kernel_spmd

    nc = _get_nc()
    in_maps = _host_prep(inputs)
    res = run_bass_kernel_spmd(
        nc, in_maps, core_ids=list(range(NCORES)), trace=trace
    )
    _cache["last_result"] = res
    return _unshard(res.results, inputs)


def kernel(**inputs):
    mask = np.asarray(inputs["attention_mask"])
    if not np.all(mask == 1):
        return _reference_numpy(inputs)
    return _run(inputs, trace=False)


def kernel_traced(**inputs):
    """Like kernel() but with NTFF profiling; returns (out, exec_time_ns)."""
    out = _run(inputs, trace=True)
    return out, _cache["last_result"].exec_time_ns

